# revision 1
# baseline (speedup 1.0000x reference)
"""Trainium2 Bass kernel for OldNeighborhoodEncoder (segment_reduce).

Math (reference):
    fc1    = relu(X @ W1.T + b1)            # [N, 64], X = [N, 3]
    pooled = segment_max(fc1, cluster, S)   # [S, 64], cluster = arange(N)//32
    h      = relu(pooled @ W1g.T + b1g)     # [S, 64]
    out    = relu(h @ W2g.T + b2g)          # [S, 128]

Hardcoded sizes: N=1048576, S=32768 (32 pts/cluster), FEATURE=64, FG0=64,
FG1=128, 8 cores. Data-parallel over points: core d handles points
[d*131072, (d+1)*131072) == clusters [d*4096, (d+1)*4096); no collectives.

Device layout (per core):
  xt [6, 65536]: col c = 512*g + o (g in 0..127, o in 0..511); rows 0-2 =
    xyz of point 1024*g + o, rows 3-5 = xyz of point 1024*g + 512 + o.
  wpack [6,128] = blockdiag(W1.T, W1.T): one matmul column-block computes
    fc1 (pre-bias) for TWO 512-point chunks at once -> full 128-partition
    PE output. Bias+relu are deferred past the max (monotone).
  psum [128,4,16,32]: bank b holds g = 4i+b; view [.., q, t] with o=32q+t,
    so a single DVE reduce over t pools 4*16 = 64 cluster-halves.
  pooled [128, 32, 4, 16]: pooled[64a+f, i, b, q] = max_z of cluster
    128i + 32b + 16a + q, feature f.
  Tail: relu(+b1) -> blockdiag(W1g.T) matmul -> relu(+b1g) ->
    W2g.T matmul (K=64, separately for a=0 from partitions 0:64 and a=1
    from 64:128) -> relu(+b2g) -> outA/outB [128, 2048].

v1.5 perf structure: the main loop is DVE-reduce-bound (Pool/GPSIMD has no
legal max op on this target, so DVE does all 32 chunk reductions); weight
DMAs go on the Scalar queue (HWDGE; gpsimd SWDGE blocked the first matmul
~7us); relu(+b1) of pooled happens in slices during the main loop on ACT;
the tail MLP is pipelined in 512-col sub-slices with relu work split
between ACT and DVE, and output DMAs are split in halves on two queues.
"""

import sys
import numpy as np

if "/opt/trn_rl_repo" not in sys.path:
    sys.path.insert(0, "/opt/trn_rl_repo")

N = 1048576
S = 32768
PTS_PER_CLUSTER = 32
FEATURE = 64
FG0 = 64
FG1 = 128
NCORES = 8
NPC = N // NCORES          # 131072 points per core
SPC = S // NCORES          # 4096 clusters per core
G = NPC // 1024            # 128 column-groups of 512
NCHUNK = 32                # psum chunks per core (each = 4 groups)

USE_F32R = True

_PROGRAM = None  # (nc, input_names) cache


def _build_program():
    from concourse import bacc, bass, tile

    mybir = bass.mybir
    f32 = mybir.dt.float32
    # float32r: fp32 bits, full-rate (1 cycle/row) PE mode. The BIR verifier
    # requires every producer of an f32r matmul operand to emit f32r, so the
    # DRAM tensors / SBUF tiles on matmul paths are declared f32r outright.
    fmm = mybir.dt.float32r if USE_F32R else f32
    AX = mybir.AxisListType

    nc = bacc.Bacc("TRN2", target_bir_lowering=False, debug=False)

    xt = nc.dram_tensor("xt", [6, G * 512], fmm, kind="ExternalInput").ap()
    wpack = nc.dram_tensor("wpack", [6, 128], fmm, kind="ExternalInput").ap()
    b1d = nc.dram_tensor("b1d", [128, 1], f32, kind="ExternalInput").ap()
    w1gbd = nc.dram_tensor("w1gbd", [128, 128], fmm, kind="ExternalInput").ap()
    b1gd = nc.dram_tensor("b1gd", [128, 1], f32, kind="ExternalInput").ap()
    w2gt = nc.dram_tensor("w2gt", [128, 128], fmm, kind="ExternalInput").ap()
    b2g = nc.dram_tensor("b2g", [128, 1], f32, kind="ExternalInput").ap()
    outA = nc.dram_tensor("outA", [128, 2048], f32, kind="ExternalOutput").ap()
    outB = nc.dram_tensor("outB", [128, 2048], f32, kind="ExternalOutput").ap()

    with tile.TileContext(nc) as tc:
        with (
            tc.tile_pool(name="w", bufs=1) as wp,
            tc.tile_pool(name="x", bufs=3) as xp,
            tc.tile_pool(name="acc", bufs=1) as accp,
            tc.tile_pool(name="ps", bufs=2, space=bass.MemorySpace.PSUM) as pp,
        ):
            wpack_t = wp.tile([6, 128], fmm, tag="wpack")
            b1d_t = wp.tile([128, 1], f32, tag="b1d")
            w1gbd_t = wp.tile([128, 128], fmm, tag="w1gbd")
            b1gd_t = wp.tile([128, 1], f32, tag="b1gd")
            w2gt_t = wp.tile([128, 128], fmm, tag="w2gt")
            b2g_t = wp.tile([128, 1], f32, tag="b2g")
            # weight DMAs on the Scalar queue (HWDGE); wpack first — it
            # gates the first matmul.
            for t, d in (
                (wpack_t, wpack),
                (b1d_t, b1d),
                (w1gbd_t, w1gbd),
                (b1gd_t, b1gd),
                (w2gt_t, w2gt),
                (b2g_t, b2g),
            ):
                nc.scalar.dma_start(t[:], d[:])

            pooled = accp.tile([128, NCHUNK, 4, 16], f32, tag="pooled")
            pooledR = accp.tile([128, 2048], fmm, tag="pooledR")

            # main loop: fc1 matmuls + segment-max pooling
            for k in range(8):  # 8 DMA chunks of [6, 8192]
                xt_t = xp.tile([6, 8192], fmm, tag="xt")
                if k == 0:
                    # split so the first matmul's columns land early
                    nc.sync.dma_start(xt_t[:, 0:2048], xt[:, 0:2048])
                    nc.sync.dma_start(xt_t[:, 2048:8192], xt[:, 2048:8192])
                else:
                    nc.sync.dma_start(xt_t[:], xt[:, k * 8192 : (k + 1) * 8192])
                for m in range(4):
                    i = 4 * k + m
                    ps = pp.tile([128, 4, 16, 32], f32, tag="ps")
                    for b in range(4):
                        c0 = (4 * m + b) * 512
                        nc.tensor.matmul(
                            ps[:, b],
                            wpack_t[:],
                            xt_t[:, c0 : c0 + 512],
                        )
                    # all reduces on DVE: it is the only engine with a
                    # free-axis max on this target (Pool/GPSIMD rejects
                    # TensorTensor/TensorReduce/InstPool at the ISA level)
                    nc.vector.reduce_max(pooled[:, i], ps[:], axis=AX.X)
                    if i % 8 == 2 and i > 8:
                        # relu(+b1) an eighth of pooled once its chunks are
                        # done; deferred two chunks so the ACT-queue wait
                        # can't stall the next eviction.
                        s = i // 8 - 1
                        nc.scalar.activation(
                            pooledR[:, s * 512 : (s + 1) * 512],
                            pooled[:, s * 8 : (s + 1) * 8],
                            mybir.ActivationFunctionType.Relu,
                            bias=b1d_t[:],
                        )

            # last eighth of pooledR
            nc.scalar.activation(
                pooledR[:, 1536:2048],
                pooled[:, 24:32],
                mybir.ActivationFunctionType.Relu,
                bias=b1d_t[:],
            )

            # tail MLP, pipelined in 512-col sub-slices
            hps = pp.tile([128, 4, 16, 32], f32, tag="ps")
            hR = accp.tile([128, 2048], fmm, tag="hR")
            for j in range(4):
                nc.tensor.matmul(
                    hps[:, j],
                    w1gbd_t[:],
                    pooledR[:, j * 512 : (j + 1) * 512],
                )
                nc.scalar.activation(
                    hR[:, j * 512 : (j + 1) * 512],
                    hps[:, j],
                    mybir.ActivationFunctionType.Relu,
                    bias=b1gd_t[:],
                )

            opsA = pp.tile([128, 4, 16, 32], f32, tag="ps")
            opsB = pp.tile([128, 4, 16, 32], f32, tag="ps")
            o2A = accp.tile([128, 2048], f32, tag="o2A")
            o2B = accp.tile([128, 2048], f32, tag="o2B")
            add = mybir.AluOpType.add
            vmax = mybir.AluOpType.max
            for j in range(4):
                nc.tensor.matmul(
                    opsA[:, j],
                    w2gt_t[0:64, :],
                    hR[0:64, j * 512 : (j + 1) * 512],
                )
                nc.tensor.matmul(
                    opsB[:, j],
                    w2gt_t[64:128, :],
                    hR[64:128, j * 512 : (j + 1) * 512],
                )
                # relu(+b2g): o2A + first half of o2B on DVE, rest on ACT
                nc.vector.tensor_scalar(
                    o2A[:, j * 512 : (j + 1) * 512],
                    opsA[:, j], b2g_t[:], 0.0, op0=add, op1=vmax,
                )
                if j < 2:
                    nc.vector.tensor_scalar(
                        o2B[:, j * 512 : (j + 1) * 512],
                        opsB[:, j], b2g_t[:], 0.0, op0=add, op1=vmax,
                    )
                else:
                    nc.scalar.activation(
                        o2B[:, j * 512 : (j + 1) * 512],
                        opsB[:, j],
                        mybir.ActivationFunctionType.Relu,
                        bias=b2g_t[:],
                    )
                if j == 1:
                    nc.sync.dma_start(outA[:, 0:1024], o2A[:, 0:1024])
                if j == 2:
                    # after the j==2 ACT so the issue's wait on DVE's
                    # o2B slices can't stall ACT compute
                    nc.scalar.dma_start(outB[:, 0:1024], o2B[:, 0:1024])
            nc.sync.dma_start(outA[:, 1024:2048], o2A[:, 1024:2048])
            nc.scalar.dma_start(outB[:, 1024:2048], o2B[:, 1024:2048])

    nc.compile()
    return nc


def _get_program():
    global _PROGRAM
    if _PROGRAM is None:
        _PROGRAM = _build_program()
    return _PROGRAM


def _host_pack(relative_points, W1, b1, W1g, b1g, W2g, b2g):
    X = np.ascontiguousarray(relative_points, dtype=np.float32)
    W1 = np.asarray(W1, np.float32)
    b1 = np.asarray(b1, np.float32)
    W1g = np.asarray(W1g, np.float32)
    b1g = np.asarray(b1g, np.float32)
    W2g = np.asarray(W2g, np.float32)
    b2g = np.asarray(b2g, np.float32)

    wpack = np.zeros((6, 128), np.float32)
    wpack[0:3, 0:64] = W1.T
    wpack[3:6, 64:128] = W1.T
    b1d = np.concatenate([b1, b1]).reshape(128, 1)
    w1gbd = np.zeros((128, 128), np.float32)
    w1gbd[0:64, 0:64] = W1g.T
    w1gbd[64:128, 64:128] = W1g.T
    b1gd = np.concatenate([b1g, b1g]).reshape(128, 1)
    w2gt = np.ascontiguousarray(np.vstack([W2g.T, W2g.T]))  # [128, 128]
    b2gc = np.ascontiguousarray(b2g.reshape(128, 1))

    in_maps = []
    for d in range(NCORES):
        Xc = X[d * NPC : (d + 1) * NPC]
        xt6 = np.ascontiguousarray(
            Xc.reshape(G, 2, 512, 3).transpose(1, 3, 0, 2).reshape(6, G * 512)
        )
        in_maps.append(
            {
                "xt": xt6,
                "wpack": wpack,
                "b1d": b1d,
                "w1gbd": w1gbd,
                "b1gd": b1gd,
                "w2gt": w2gt,
                "b2g": b2gc,
            }
        )
    return in_maps


def _host_unpack(results):
    out = np.empty((S, FG1), np.float32)
    for d in range(NCORES):
        oA = results[d]["outA"].reshape(128, NCHUNK, 4, 16)
        oB = results[d]["outB"].reshape(128, NCHUNK, 4, 16)
        blk = out[d * SPC : (d + 1) * SPC].reshape(NCHUNK, 4, 2, 16, 128)
        blk[:, :, 0] = oA.transpose(1, 2, 3, 0)
        blk[:, :, 1] = oB.transpose(1, 2, 3, 0)
    return out


def _numpy_fallback(relative_points, cluster, num_clusters,
                    W1, b1, W1g, b1g, W2g, b2g):
    X = np.asarray(relative_points, np.float32)
    fc1 = np.maximum(X @ np.asarray(W1, np.float32).T + np.asarray(b1, np.float32), 0.0)
    Sn = int(num_clusters)
    cl = np.asarray(cluster).astype(np.int64)
    pooled = np.full((Sn, fc1.shape[1]), -np.inf, np.float32)
    # sorted segment ids -> reduceat over run starts
    starts = np.flatnonzero(np.r_[True, cl[1:] != cl[:-1]])
    seg_ids = cl[starts]
    pooled[seg_ids] = np.maximum.reduceat(fc1, starts, axis=0)
    h = np.maximum(pooled @ np.asarray(W1g, np.float32).T + np.asarray(b1g, np.float32), 0.0)
    return np.maximum(h @ np.asarray(W2g, np.float32).T + np.asarray(b2g, np.float32), 0.0).astype(np.float32)


def _run_hw(in_maps, trace=False):
    from concourse.bass_utils import run_bass_kernel_spmd

    nc = _get_program()
    return run_bass_kernel_spmd(
        nc, in_maps, list(range(NCORES)), trace=trace
    )


def kernel(relative_points, cluster, num_clusters,
           W1, b1, W1g, b1g, W2g, b2g):
    cl = np.asarray(cluster)
    expected_cl = np.arange(N, dtype=np.int64) // PTS_PER_CLUSTER
    if (
        relative_points.shape != (N, 3)
        or int(num_clusters) != S
        or not np.array_equal(cl, expected_cl)
    ):
        return _numpy_fallback(relative_points, cluster, num_clusters,
                               W1, b1, W1g, b1g, W2g, b2g)

    in_maps = _host_pack(relative_points, W1, b1, W1g, b1g, W2g, b2g)
    res = _run_hw(in_maps, trace=False)
    return _host_unpack(res.results)


def run_traced(inputs):
    """test.py helper: returns (output, exec_time_ns)."""
    in_maps = _host_pack(
        inputs["relative_points"], inputs["W1"], inputs["b1"],
        inputs["W1g"], inputs["b1g"], inputs["W2g"], inputs["b2g"],
    )
    res = _run_hw(in_maps, trace=True)
    return _host_unpack(res.results), res.exec_time_ns



# revision 2
# speedup vs baseline: 1.0937x; 1.0937x over previous
"""Trainium2 Bass kernel for OldNeighborhoodEncoder (segment_reduce).

Math (reference):
    fc1    = relu(X @ W1.T + b1)            # [N, 64], X = [N, 3]
    pooled = segment_max(fc1, cluster, S)   # [S, 64], cluster = arange(N)//32
    h      = relu(pooled @ W1g.T + b1g)     # [S, 64]
    out    = relu(h @ W2g.T + b2g)          # [S, 128]

Hardcoded sizes: N=1048576, S=32768 (32 pts/cluster), FEATURE=64, FG0=64,
FG1=128, 8 cores. Data-parallel over points: core d handles points
[d*131072, (d+1)*131072) == clusters [d*4096, (d+1)*4096); no collectives.

v2 design notes (per core):
  - All matmul inputs bf16 (halves input DMA, 1 cyc/row PE). PSUM stays f32.
  - xt [6, 65536] bf16: col c = 512*g + o; rows 0-2 = xyz of point 1024g+o,
    rows 3-5 = xyz of point 1024g+512+o. wpack [6,128] = blockdiag(W1.T x2):
    one column computes fc1 (pre-bias) for TWO points -> 128 PE partitions.
  - Main loop: 43 psum chunks of 3 banks ([128, 3, 16, 32] = 1536 cols;
    last chunk 2 banks). The 32->1 segment max is the bottleneck: DVE
    reduce_max from PSUM runs at 1 elem/cycle/lane (tensor_reduce has only
    a 1x uop), so chunks are split between two PSUM read ports:
      A-chunks: ACT does relu(+b1) copy PSUM->SBUF as bf16 (1x @1.2GHz),
        then DVE runs a 5-level tensor_max tree on SBUF bf16 at 2x_1P mode
        (2 results/cycle, 4 elems/cycle ingest) -> ~1.07us/chunk DVE.
      D-chunks: DVE reduce_max straight from PSUM (~1.8us/chunk), relu(+b1)
        applied after via a cheap 4x tensor_scalar on the [128,48] pooled.
    Split ~34 A / 9 D balances ACT ~54us and DVE ~55us busy.
  - Tail MLP pipelined INTO the main loop as 512-col slices (after chunks
    10/21/31/42) using a dedicated 1-bank psum tile; relu work split
    ACT/DVE; outputs DMA'd out as they complete (striped across hw queues).
  - PSUM budget: 2x3 banks main double-buffer + 1 tail + 1 spare = 8.
"""

import sys
import numpy as np
import ml_dtypes

if "/opt/trn_rl_repo" not in sys.path:
    sys.path.insert(0, "/opt/trn_rl_repo")

BF16 = ml_dtypes.bfloat16

N = 1048576
S = 32768
PTS_PER_CLUSTER = 32
FEATURE = 64
FG0 = 64
FG1 = 128
NCORES = 8
NPC = N // NCORES          # 131072 points per core
SPC = S // NCORES          # 4096 clusters per core
G = NPC // 1024            # 128 column-groups of 512

# main-loop chunking: 42 chunks of 3 psum banks + 1 of 2 banks = 128 groups
NCHUNKS = 43
BANKS = [3] * 42 + [2]
CHUNK_G0 = [3 * c for c in range(NCHUNKS)]      # first group of chunk c
# chunks whose psum is drained by DVE reduce_max directly (D-chunks)
D_CHUNKS = {4, 8, 13, 18, 23, 28, 33, 38, 41}
# tail slice j (512 pooled cols) emitted after this chunk
TAIL_AFTER = {10: 0, 21: 1, 31: 2, 42: 3}

_PROGRAM = None


def _build_program():
    from concourse import bacc, bass, tile

    mybir = bass.mybir
    f32 = mybir.dt.float32
    bf16 = mybir.dt.bfloat16
    AX = mybir.AxisListType
    add = mybir.AluOpType.add
    vmax = mybir.AluOpType.max
    Relu = mybir.ActivationFunctionType.Relu

    nc = bacc.Bacc("TRN2", target_bir_lowering=False, debug=False)

    xt = nc.dram_tensor("xt", [6, G * 512], bf16, kind="ExternalInput").ap()
    wpack = nc.dram_tensor("wpack", [6, 128], bf16, kind="ExternalInput").ap()
    b1d = nc.dram_tensor("b1d", [128, 1], f32, kind="ExternalInput").ap()
    w1gbd = nc.dram_tensor("w1gbd", [128, 128], bf16, kind="ExternalInput").ap()
    b1gd = nc.dram_tensor("b1gd", [128, 1], f32, kind="ExternalInput").ap()
    w2gt = nc.dram_tensor("w2gt", [128, 128], bf16, kind="ExternalInput").ap()
    b2g = nc.dram_tensor("b2g", [128, 1], f32, kind="ExternalInput").ap()
    outA = nc.dram_tensor("outA", [128, 2048], f32, kind="ExternalOutput").ap()
    outB = nc.dram_tensor("outB", [128, 2048], f32, kind="ExternalOutput").ap()

    with tile.TileContext(nc) as tc:
        with (
            tc.tile_pool(name="w", bufs=1) as wp,
            tc.tile_pool(name="x", bufs=6) as xp,
            tc.tile_pool(name="sb", bufs=3) as sp,
            tc.tile_pool(name="tr", bufs=2) as trp,
            tc.tile_pool(name="pd", bufs=2) as pdp,
            tc.tile_pool(name="hr", bufs=2) as hrp,
            tc.tile_pool(name="acc", bufs=1) as accp,
            tc.tile_pool(name="ps", bufs=2, space=bass.MemorySpace.PSUM) as pp,
            tc.tile_pool(name="tps", bufs=1, space=bass.MemorySpace.PSUM) as tpp,
        ):
            wpack_t = wp.tile([6, 128], bf16, tag="wpack")
            b1d_t = wp.tile([128, 1], f32, tag="b1d")
            w1gbd_t = wp.tile([128, 128], bf16, tag="w1gbd")
            b1gd_t = wp.tile([128, 1], f32, tag="b1gd")
            w2gt_t = wp.tile([128, 128], bf16, tag="w2gt")
            b2g_t = wp.tile([128, 1], f32, tag="b2g")
            for t, d in (
                (wpack_t, wpack),
                (b1d_t, b1d),
                (w1gbd_t, w1gbd),
                (b1gd_t, b1gd),
                (w2gt_t, w2gt),
                (b2g_t, b2g),
            ):
                nc.scalar.dma_start(t[:], d[:])

            pooledR = accp.tile([128, 2048], bf16, tag="pooledR")
            o2A = accp.tile([128, 2048], f32, tag="o2A")
            o2B = accp.tile([128, 2048], f32, tag="o2B")

            tps = tpp.tile([128, 512], f32, tag="tps")

            # tail state carried across the interleaved stages
            tail_live = {}  # j -> hR tile

            def tail_stage1(j):
                # mm1 + relu(+b1g) -> hR  (pooledR cols 512j..512j+512 ready)
                nc.tensor.matmul(
                    tps[:], w1gbd_t[:], pooledR[:, 512 * j : 512 * (j + 1)]
                )
                hR = hrp.tile([128, 512], bf16, tag="hR")
                nc.scalar.activation(hR[:], tps[:], Relu, bias=b1gd_t[:])
                tail_live[j] = hR

            def tail_stage2(j):
                hR = tail_live.pop(j)
                lo, hi = 512 * j, 512 * (j + 1)
                nc.tensor.matmul(tps[:], w2gt_t[0:64, :], hR[0:64, :])
                # relu(+b2g): A on DVE, B on ACT
                nc.vector.tensor_scalar(
                    o2A[:, lo:hi], tps[:], b2g_t[:], 0.0, op0=add, op1=vmax
                )
                nc.tensor.matmul(tps[:], w2gt_t[64:128, :], hR[64:128, :])
                nc.scalar.activation(
                    o2B[:, lo:hi], tps[:], Relu, bias=b2g_t[:]
                )
                nc.sync.dma_start(outA[:, lo:hi], o2A[:, lo:hi])
                nc.sync.dma_start(outB[:, lo:hi], o2B[:, lo:hi])

            pending_stage2 = []

            for c in range(NCHUNKS):
                nb = BANKS[c]
                cols = 512 * nb
                pc = 16 * nb                      # pooled cols this chunk
                x0 = 512 * CHUNK_G0[c]            # xt col offset
                p0 = 16 * CHUNK_G0[c]             # pooledR col offset

                xt_t = xp.tile([6, 1536], bf16, tag="xt")
                if c == 0:
                    nc.sync.dma_start(xt_t[:, 0:512], xt[:, 0:512])
                    nc.sync.dma_start(xt_t[:, 512:cols], xt[:, 512 : x0 + cols])
                else:
                    nc.sync.dma_start(xt_t[:, 0:cols], xt[:, x0 : x0 + cols])

                ps = pp.tile([128, 3, 16, 32], f32, tag="ps")
                for b in range(nb):
                    nc.tensor.matmul(
                        ps[:, b], wpack_t[:], xt_t[:, 512 * b : 512 * (b + 1)]
                    )

                # stage-2 of a pending tail slice rides one chunk later so
                # its ACT/DVE deps are a full chunk old (no engine stalls)
                if pending_stage2:
                    tail_stage2(pending_stage2.pop())

                if c in D_CHUNKS:
                    pd = pdp.tile([128, 48], f32, tag="pd")
                    nc.vector.reduce_max(
                        pd[:, 0:pc].rearrange(f"p (b q) -> p b q", b=nb),
                        ps[:, 0:nb],
                        axis=AX.X,
                    )
                    nc.vector.tensor_scalar(
                        pooledR[:, p0 : p0 + pc],
                        pd[:, 0:pc],
                        b1d_t[:],
                        0.0,
                        op0=add,
                        op1=vmax,
                    )
                else:
                    sb = sp.tile([128, 1536], bf16, tag="sbc")
                    nc.scalar.activation(
                        sb[:, 0:cols], ps[:, 0:nb], Relu, bias=b1d_t[:]
                    )
                    sv = sb[:, 0:cols].rearrange("p (r t) -> p r t", t=32)
                    y1 = trp.tile([128, 48, 16], bf16, tag="y1")
                    y2 = trp.tile([128, 48, 8], bf16, tag="y2")
                    y3 = trp.tile([128, 48, 4], bf16, tag="y3")
                    y4 = trp.tile([128, 48, 2], bf16, tag="y4")
                    r = 16 * nb
                    nc.vector.tensor_max(
                        y1[:, 0:r], sv[:, :, 0:16], sv[:, :, 16:32]
                    )
                    nc.vector.tensor_max(
                        y2[:, 0:r], y1[:, 0:r, 0:8], y1[:, 0:r, 8:16]
                    )
                    nc.vector.tensor_max(
                        y3[:, 0:r], y2[:, 0:r, 0:4], y2[:, 0:r, 4:8]
                    )
                    nc.vector.tensor_max(
                        y4[:, 0:r], y3[:, 0:r, 0:2], y3[:, 0:r, 2:4]
                    )
                    nc.vector.tensor_max(
                        pooledR[:, p0 : p0 + pc],
                        y4[:, 0:r, 0],
                        y4[:, 0:r, 1],
                    )

                if c in TAIL_AFTER:
                    j = TAIL_AFTER[c]
                    tail_stage1(j)
                    if c == 42:
                        tail_stage2(j)
                    else:
                        pending_stage2.append(j)

    nc.compile()
    return nc


def _get_program():
    global _PROGRAM
    if _PROGRAM is None:
        _PROGRAM = _build_program()
    return _PROGRAM


def _host_pack(relative_points, W1, b1, W1g, b1g, W2g, b2g):
    X = np.ascontiguousarray(relative_points, dtype=np.float32)
    W1 = np.asarray(W1, np.float32)
    b1 = np.asarray(b1, np.float32)
    W1g = np.asarray(W1g, np.float32)
    b1g = np.asarray(b1g, np.float32)
    W2g = np.asarray(W2g, np.float32)
    b2g = np.asarray(b2g, np.float32)

    wpack = np.zeros((6, 128), np.float32)
    wpack[0:3, 0:64] = W1.T
    wpack[3:6, 64:128] = W1.T
    b1d = np.concatenate([b1, b1]).reshape(128, 1)
    w1gbd = np.zeros((128, 128), np.float32)
    w1gbd[0:64, 0:64] = W1g.T
    w1gbd[64:128, 64:128] = W1g.T
    b1gd = np.concatenate([b1g, b1g]).reshape(128, 1)
    w2gt = np.vstack([W2g.T, W2g.T])  # [128, 128]
    b2gc = np.ascontiguousarray(b2g.reshape(128, 1))

    wpack = wpack.astype(BF16)
    w1gbd = w1gbd.astype(BF16)
    w2gt = np.ascontiguousarray(w2gt.astype(BF16))

    in_maps = []
    for d in range(NCORES):
        Xc = X[d * NPC : (d + 1) * NPC]
        xt6 = np.ascontiguousarray(
            Xc.reshape(G, 2, 512, 3)
            .transpose(1, 3, 0, 2)
            .reshape(6, G * 512)
            .astype(BF16)
        )
        in_maps.append(
            {
                "xt": xt6,
                "wpack": wpack,
                "b1d": b1d,
                "w1gbd": w1gbd,
                "b1gd": b1gd,
                "w2gt": w2gt,
                "b2g": b2gc,
            }
        )
    return in_maps


def _host_unpack(results):
    out = np.empty((S, FG1), np.float32)
    for d in range(NCORES):
        oA = results[d]["outA"].reshape(128, 32, 4, 16)
        oB = results[d]["outB"].reshape(128, 32, 4, 16)
        blk = out[d * SPC : (d + 1) * SPC].reshape(32, 4, 2, 16, 128)
        blk[:, :, 0] = oA.transpose(1, 2, 3, 0)
        blk[:, :, 1] = oB.transpose(1, 2, 3, 0)
    return out


def _numpy_fallback(relative_points, cluster, num_clusters,
                    W1, b1, W1g, b1g, W2g, b2g):
    X = np.asarray(relative_points, np.float32)
    fc1 = np.maximum(X @ np.asarray(W1, np.float32).T + np.asarray(b1, np.float32), 0.0)
    Sn = int(num_clusters)
    cl = np.asarray(cluster).astype(np.int64)
    pooled = np.full((Sn, fc1.shape[1]), -np.inf, np.float32)
    starts = np.flatnonzero(np.r_[True, cl[1:] != cl[:-1]])
    seg_ids = cl[starts]
    pooled[seg_ids] = np.maximum.reduceat(fc1, starts, axis=0)
    h = np.maximum(pooled @ np.asarray(W1g, np.float32).T + np.asarray(b1g, np.float32), 0.0)
    return np.maximum(h @ np.asarray(W2g, np.float32).T + np.asarray(b2g, np.float32), 0.0).astype(np.float32)


def _run_hw(in_maps, trace=False):
    from concourse.bass_utils import run_bass_kernel_spmd

    nc = _get_program()
    return run_bass_kernel_spmd(
        nc, in_maps, list(range(NCORES)), trace=trace
    )


def kernel(relative_points, cluster, num_clusters,
           W1, b1, W1g, b1g, W2g, b2g):
    cl = np.asarray(cluster)
    expected_cl = np.arange(N, dtype=np.int64) // PTS_PER_CLUSTER
    if (
        relative_points.shape != (N, 3)
        or int(num_clusters) != S
        or not np.array_equal(cl, expected_cl)
    ):
        return _numpy_fallback(relative_points, cluster, num_clusters,
                               W1, b1, W1g, b1g, W2g, b2g)

    in_maps = _host_pack(relative_points, W1, b1, W1g, b1g, W2g, b2g)
    res = _run_hw(in_maps, trace=False)
    return _host_unpack(res.results)


def run_traced(inputs):
    """test.py helper: returns (output, exec_time_ns)."""
    in_maps = _host_pack(
        inputs["relative_points"], inputs["W1"], inputs["b1"],
        inputs["W1g"], inputs["b1g"], inputs["W2g"], inputs["b2g"],
    )
    res = _run_hw(in_maps, trace=True)
    return _host_unpack(res.results), res.exec_time_ns


# revision 3
# speedup vs baseline: 1.1166x; 1.0209x over previous
"""Trainium2 Bass kernel for OldNeighborhoodEncoder (segment_reduce).

Math (reference):
    fc1    = relu(X @ W1.T + b1)            # [N, 64], X = [N, 3]
    pooled = segment_max(fc1, cluster, S)   # [S, 64], cluster = arange(N)//32
    h      = relu(pooled @ W1g.T + b1g)     # [S, 64]
    out    = relu(h @ W2g.T + b2g)          # [S, 128]

Hardcoded sizes: N=1048576, S=32768 (32 pts/cluster), 8 cores; core d does
points [d*131072, (d+1)*131072) == clusters [d*4096, (d+1)*4096).

v3 design (per core). Measured HW facts this is built around: PE is pinned
at 1.2 GHz (no p-state ramp; 512-col matmul = 427ns, exactly FD*0.833ns),
DVE tensor_reduce from PSUM = 1 elem/cyc/lane @0.96GHz, ACT activation =
1x @1.2GHz, DVE tensor_max on SBUF bf16 hits 2x_1P (2 results/cyc).

  - bf16 everywhere except PSUM/bias/output. xt [6, 65536]: col c = 512g+o;
    rows 0-2 = xyz of point 1024g+o, rows 3-5 = xyz of point 1024g+512+o;
    wpack [6,128] = blockdiag(W1.T x2) -> one col = fc1 of TWO points.
  - 43 psum chunks of 3 banks ([128, 3, 16, 32]; last 2 banks). Drains are
    split across the two PSUM read ports to keep pace with PE:
      A-chunks (31): ACT relu(+b1)-copies PSUM->SBUF bf16 (~1.66us);
        DVE later runs a tensor_max tree on the copy at 2x. Trees are
        DEFERRED (emitted 1+ chunks later) and BATCHED in pairs (L1-L4
        over 3072 cols, two per-chunk L5s) so DVE ops never gate the
        psum ping-pong or the ACT copy cadence.
      D-chunks (12): DVE reduce_max straight from PSUM + a cheap 4x
        tensor_scalar relu(+b1) on the [128,48] pooled slice.
    SBUF copies land in a 4-slot rotating buffer (one big tile) so pair
    batching works regardless of interleaving.
  - Tail MLP pipelined into the loop in slices (512,512,512,256,256 cols),
    two dedicated psum banks so mm2A/mm2B run back-to-back; relu work
    split DVE/ACT; slice DMAs stripe over all 16 hw queues as they
    complete. Only the last 256-col slice trails the loop.
  - PSUM: 2x3 banks main + 2 tail = 8.
"""

import sys
import numpy as np
import ml_dtypes

if "/opt/trn_rl_repo" not in sys.path:
    sys.path.insert(0, "/opt/trn_rl_repo")

BF16 = ml_dtypes.bfloat16

N = 1048576
S = 32768
PTS_PER_CLUSTER = 32
FEATURE = 64
FG0 = 64
FG1 = 128
NCORES = 8
NPC = N // NCORES          # 131072 points per core
SPC = S // NCORES          # 4096 clusters per core
G = NPC // 1024            # 128 column-groups of 512

NCHUNKS = 43
BANKS = [3] * 42 + [2]
D_CHUNKS = {3, 6, 9, 14, 17, 20, 25, 28, 33, 36, 39, 41}
# block -> (tail slice id, col lo, col hi); stage1 emitted at block key,
# stage2 two blocks later (inline at the end for the last slice)
TAIL_SLICES = {11: (0, 0, 512), 22: (1, 512, 1024), 32: (2, 1024, 1536),
               38: (3, 1536, 1792)}
FLUSH_AT = {10, 21, 31, 37}   # pop all queued trees before these stage1s

_PROGRAM = None


def _build_program():
    from concourse import bacc, bass, tile

    mybir = bass.mybir
    f32 = mybir.dt.float32
    bf16 = mybir.dt.bfloat16
    AX = mybir.AxisListType
    add = mybir.AluOpType.add
    vmax = mybir.AluOpType.max
    Relu = mybir.ActivationFunctionType.Relu

    nc = bacc.Bacc("TRN2", target_bir_lowering=False, debug=False)

    xt = nc.dram_tensor("xt", [6, G * 512], bf16, kind="ExternalInput").ap()
    wpack = nc.dram_tensor("wpack", [6, 128], bf16, kind="ExternalInput").ap()
    b1d = nc.dram_tensor("b1d", [128, 1], f32, kind="ExternalInput").ap()
    w1gbd = nc.dram_tensor("w1gbd", [128, 128], bf16, kind="ExternalInput").ap()
    b1gd = nc.dram_tensor("b1gd", [128, 1], f32, kind="ExternalInput").ap()
    w2gt = nc.dram_tensor("w2gt", [128, 128], bf16, kind="ExternalInput").ap()
    b2g = nc.dram_tensor("b2g", [128, 1], f32, kind="ExternalInput").ap()
    outA = nc.dram_tensor("outA", [128, 2048], f32, kind="ExternalOutput").ap()
    outB = nc.dram_tensor("outB", [128, 2048], f32, kind="ExternalOutput").ap()

    with tile.TileContext(nc) as tc:
        with (
            tc.tile_pool(name="w", bufs=1) as wp,
            tc.tile_pool(name="x", bufs=6) as xp,
            tc.tile_pool(name="tr", bufs=2) as trp,
            tc.tile_pool(name="pd", bufs=2) as pdp,
            tc.tile_pool(name="hr", bufs=2) as hrp,
            tc.tile_pool(name="acc", bufs=1) as accp,
            tc.tile_pool(name="ps", bufs=2, space=bass.MemorySpace.PSUM) as pp,
            tc.tile_pool(name="tpa", bufs=1, space=bass.MemorySpace.PSUM) as tpa,
            tc.tile_pool(name="tpb", bufs=1, space=bass.MemorySpace.PSUM) as tpb,
        ):
            wpack_t = wp.tile([6, 128], bf16, tag="wpack")
            b1d_t = wp.tile([128, 1], f32, tag="b1d")
            w1gbd_t = wp.tile([128, 128], bf16, tag="w1gbd")
            b1gd_t = wp.tile([128, 1], f32, tag="b1gd")
            w2gt_t = wp.tile([128, 128], bf16, tag="w2gt")
            b2g_t = wp.tile([128, 1], f32, tag="b2g")
            for t, d in (
                (wpack_t, wpack),
                (b1d_t, b1d),
                (w1gbd_t, w1gbd),
                (b1gd_t, b1gd),
                (w2gt_t, w2gt),
                (b2g_t, b2g),
            ):
                nc.scalar.dma_start(t[:], d[:])

            pooledR = accp.tile([128, 2048], bf16, tag="pooledR")
            o2A = accp.tile([128, 2048], f32, tag="o2A")
            o2B = accp.tile([128, 2048], f32, tag="o2B")
            # 4-slot rotating buffer for ACT's relu-copies (one tile so
            # adjacent slots can be tree-reduced in one batched op)
            sbbig = accp.tile([128, 4, 1536], bf16, tag="sbbig")

            tpsA = tpa.tile([128, 512], f32, tag="tpsA")
            tpsB = tpb.tile([128, 512], f32, tag="tpsB")

            tail_hr = {}

            def tail_stage1(j, lo, hi):
                w = hi - lo
                nc.tensor.matmul(tpsA[:, 0:w], w1gbd_t[:], pooledR[:, lo:hi])
                hR = hrp.tile([128, 512], bf16, tag="hR")
                nc.scalar.activation(hR[:, 0:w], tpsA[:, 0:w], Relu,
                                     bias=b1gd_t[:])
                tail_hr[j] = hR

            def tail_stage2(j, lo, hi):
                w = hi - lo
                hR = tail_hr.pop(j)
                nc.tensor.matmul(tpsA[:, 0:w], w2gt_t[0:64, :], hR[0:64, 0:w])
                nc.tensor.matmul(tpsB[:, 0:w], w2gt_t[64:128, :],
                                 hR[64:128, 0:w])
                nc.vector.tensor_scalar(
                    o2A[:, lo:hi], tpsA[:, 0:w], b2g_t[:], 0.0,
                    op0=add, op1=vmax,
                )
                nc.scalar.activation(o2B[:, lo:hi], tpsB[:, 0:w], Relu,
                                     bias=b2g_t[:])
                nc.sync.dma_start(outA[:, lo:hi], o2A[:, lo:hi])
                nc.sync.dma_start(outB[:, lo:hi], o2B[:, lo:hi])

            # deferred tree machinery
            pending = []      # [(chunk, slot, cols)] copies not yet treed
            tree_q = []       # emission thunks, popped one per block
            a_count = 0

            def emit_pair_tree(c1, s1, c2, s2):
                # batched L1-L4 over both slots (contiguous), per-chunk L5
                v = sbbig[:, s1 : s1 + 2].rearrange("p s (r t) -> p (s r) t",
                                                    t=32)
                y1 = trp.tile([128, 96, 16], bf16, tag="y1")
                y2 = trp.tile([128, 96, 8], bf16, tag="y2")
                y3 = trp.tile([128, 96, 4], bf16, tag="y3")
                y4 = trp.tile([128, 96, 2], bf16, tag="y4")
                nc.vector.tensor_max(y1[:], v[:, :, 0:16], v[:, :, 16:32])
                nc.vector.tensor_max(y2[:], y1[:, :, 0:8], y1[:, :, 8:16])
                nc.vector.tensor_max(y3[:], y2[:, :, 0:4], y2[:, :, 4:8])
                nc.vector.tensor_max(y4[:], y3[:, :, 0:2], y3[:, :, 2:4])
                for c, off in ((c1, 0), (c2, 48)):
                    p0 = 48 * c
                    nc.vector.tensor_max(
                        pooledR[:, p0 : p0 + 48],
                        y4[:, off : off + 48, 0],
                        y4[:, off : off + 48, 1],
                    )

            def emit_single_tree(c, slot, cols):
                r = cols // 32
                v = sbbig[:, slot, 0:cols].rearrange("p (r t) -> p r t", t=32)
                y1 = trp.tile([128, 96, 16], bf16, tag="y1")
                y2 = trp.tile([128, 96, 8], bf16, tag="y2")
                y3 = trp.tile([128, 96, 4], bf16, tag="y3")
                y4 = trp.tile([128, 96, 2], bf16, tag="y4")
                nc.vector.tensor_max(y1[:, 0:r], v[:, :, 0:16], v[:, :, 16:32])
                nc.vector.tensor_max(y2[:, 0:r], y1[:, 0:r, 0:8],
                                     y1[:, 0:r, 8:16])
                nc.vector.tensor_max(y3[:, 0:r], y2[:, 0:r, 0:4],
                                     y2[:, 0:r, 4:8])
                nc.vector.tensor_max(y4[:, 0:r], y3[:, 0:r, 0:2],
                                     y3[:, 0:r, 2:4])
                p0 = 48 * c
                nc.vector.tensor_max(
                    pooledR[:, p0 : p0 + r],
                    y4[:, 0:r, 0],
                    y4[:, 0:r, 1],
                )

            def queue_trees(force=False):
                while len(pending) >= 2:
                    (c1, s1, w1), (c2, s2, w2) = pending[0], pending[1]
                    if s1 % 2 == 0 and s2 == s1 + 1 and w1 == 1536 and w2 == 1536:
                        tree_q.append(lambda a=c1, b=s1, c=c2, d=s2:
                                      emit_pair_tree(a, b, c, d))
                        del pending[0:2]
                    else:
                        tree_q.append(lambda a=c1, b=s1, w=w1:
                                      emit_single_tree(a, b, w))
                        del pending[0]
                if force and pending:
                    c1, s1, w1 = pending.pop(0)
                    tree_q.append(lambda a=c1, b=s1, w=w1:
                                  emit_single_tree(a, b, w))

            for c in range(NCHUNKS):
                nb = BANKS[c]
                cols = 512 * nb
                x0 = 1536 * c

                xt_t = xp.tile([6, 1536], bf16, tag="xt")
                if c == 0:
                    nc.sync.dma_start(xt_t[:, 0:512], xt[:, 0:512])
                    nc.sync.dma_start(xt_t[:, 512:cols], xt[:, 512:cols])
                else:
                    nc.sync.dma_start(xt_t[:, 0:cols], xt[:, x0 : x0 + cols])

                ps = pp.tile([128, 3, 16, 32], f32, tag="ps")
                for b in range(nb):
                    nc.tensor.matmul(
                        ps[:, b], wpack_t[:], xt_t[:, 512 * b : 512 * (b + 1)]
                    )

                # stage2 of the tail slice started two blocks ago
                for blk, (j, lo, hi) in TAIL_SLICES.items():
                    if c == blk + 2:
                        tail_stage2(j, lo, hi)

                if c in D_CHUNKS:
                    pc = 16 * nb
                    p0 = 48 * c
                    pd = pdp.tile([128, 48], f32, tag="pd")
                    nc.vector.reduce_max(
                        pd[:, 0:pc].rearrange("p (b q) -> p b q", b=nb),
                        ps[:, 0:nb],
                        axis=AX.X,
                    )
                    nc.vector.tensor_scalar(
                        pooledR[:, p0 : p0 + pc], pd[:, 0:pc], b1d_t[:], 0.0,
                        op0=add, op1=vmax,
                    )
                else:
                    slot = a_count % 4
                    a_count += 1
                    nc.scalar.activation(
                        sbbig[:, slot, 0:cols], ps[:, 0:nb], Relu,
                        bias=b1d_t[:],
                    )
                    pending.append((c, slot, cols))
                    queue_trees()

                if c in FLUSH_AT:
                    queue_trees(force=True)
                    while tree_q:
                        tree_q.pop(0)()
                elif tree_q:
                    tree_q.pop(0)()

                if c in TAIL_SLICES:
                    tail_stage1(*TAIL_SLICES[c])

            # final chunk's tree + last tail slice (cols 1792:2048)
            queue_trees(force=True)
            while tree_q:
                tree_q.pop(0)()
            tail_stage1(4, 1792, 2048)
            tail_stage2(4, 1792, 2048)

    nc.compile()
    return nc


def _get_program():
    global _PROGRAM
    if _PROGRAM is None:
        _PROGRAM = _build_program()
    return _PROGRAM


def _host_pack(relative_points, W1, b1, W1g, b1g, W2g, b2g):
    X = np.ascontiguousarray(relative_points, dtype=np.float32)
    W1 = np.asarray(W1, np.float32)
    b1 = np.asarray(b1, np.float32)
    W1g = np.asarray(W1g, np.float32)
    b1g = np.asarray(b1g, np.float32)
    W2g = np.asarray(W2g, np.float32)
    b2g = np.asarray(b2g, np.float32)

    wpack = np.zeros((6, 128), np.float32)
    wpack[0:3, 0:64] = W1.T
    wpack[3:6, 64:128] = W1.T
    b1d = np.concatenate([b1, b1]).reshape(128, 1)
    w1gbd = np.zeros((128, 128), np.float32)
    w1gbd[0:64, 0:64] = W1g.T
    w1gbd[64:128, 64:128] = W1g.T
    b1gd = np.concatenate([b1g, b1g]).reshape(128, 1)
    w2gt = np.vstack([W2g.T, W2g.T])  # [128, 128]
    b2gc = np.ascontiguousarray(b2g.reshape(128, 1))

    wpack = wpack.astype(BF16)
    w1gbd = w1gbd.astype(BF16)
    w2gt = np.ascontiguousarray(w2gt.astype(BF16))

    in_maps = []
    for d in range(NCORES):
        Xc = X[d * NPC : (d + 1) * NPC]
        xt6 = np.ascontiguousarray(
            Xc.reshape(G, 2, 512, 3)
            .transpose(1, 3, 0, 2)
            .reshape(6, G * 512)
            .astype(BF16)
        )
        in_maps.append(
            {
                "xt": xt6,
                "wpack": wpack,
                "b1d": b1d,
                "w1gbd": w1gbd,
                "b1gd": b1gd,
                "w2gt": w2gt,
                "b2g": b2gc,
            }
        )
    return in_maps


def _host_unpack(results):
    out = np.empty((S, FG1), np.float32)
    for d in range(NCORES):
        oA = results[d]["outA"].reshape(128, 32, 4, 16)
        oB = results[d]["outB"].reshape(128, 32, 4, 16)
        blk = out[d * SPC : (d + 1) * SPC].reshape(32, 4, 2, 16, 128)
        blk[:, :, 0] = oA.transpose(1, 2, 3, 0)
        blk[:, :, 1] = oB.transpose(1, 2, 3, 0)
    return out


def _numpy_fallback(relative_points, cluster, num_clusters,
                    W1, b1, W1g, b1g, W2g, b2g):
    X = np.asarray(relative_points, np.float32)
    fc1 = np.maximum(X @ np.asarray(W1, np.float32).T + np.asarray(b1, np.float32), 0.0)
    Sn = int(num_clusters)
    cl = np.asarray(cluster).astype(np.int64)
    pooled = np.full((Sn, fc1.shape[1]), -np.inf, np.float32)
    starts = np.flatnonzero(np.r_[True, cl[1:] != cl[:-1]])
    seg_ids = cl[starts]
    pooled[seg_ids] = np.maximum.reduceat(fc1, starts, axis=0)
    h = np.maximum(pooled @ np.asarray(W1g, np.float32).T + np.asarray(b1g, np.float32), 0.0)
    return np.maximum(h @ np.asarray(W2g, np.float32).T + np.asarray(b2g, np.float32), 0.0).astype(np.float32)


def _run_hw(in_maps, trace=False):
    from concourse.bass_utils import run_bass_kernel_spmd

    nc = _get_program()
    return run_bass_kernel_spmd(
        nc, in_maps, list(range(NCORES)), trace=trace
    )


def kernel(relative_points, cluster, num_clusters,
           W1, b1, W1g, b1g, W2g, b2g):
    cl = np.asarray(cluster)
    expected_cl = np.arange(N, dtype=np.int64) // PTS_PER_CLUSTER
    if (
        relative_points.shape != (N, 3)
        or int(num_clusters) != S
        or not np.array_equal(cl, expected_cl)
    ):
        return _numpy_fallback(relative_points, cluster, num_clusters,
                               W1, b1, W1g, b1g, W2g, b2g)

    in_maps = _host_pack(relative_points, W1, b1, W1g, b1g, W2g, b2g)
    res = _run_hw(in_maps, trace=False)
    return _host_unpack(res.results)


def run_traced(inputs):
    """test.py helper: returns (output, exec_time_ns)."""
    in_maps = _host_pack(
        inputs["relative_points"], inputs["W1"], inputs["b1"],
        inputs["W1g"], inputs["b1g"], inputs["W2g"], inputs["b2g"],
    )
    res = _run_hw(in_maps, trace=True)
    return _host_unpack(res.results), res.exec_time_ns
